# revision 22
# baseline (speedup 1.0000x reference)
"""Distributed Trainium2 Bass kernel for nn_AttentionD_12412455485977.

3D-windowed multi-head attention with relative-position bias:
  qkv = x @ w_qkv ; per-head attention with bias gathered from rel_table
  via the static relative-position index; out = attn_out @ w_out + b_out.

Sharding: head-parallel. Core c computes head c for both batches; each core
returns an UNNORMALIZED projected output [C, B*N] plus the per-(b, i) softmax
denominator; the host divides and sums across cores (the natural unshard of a
head-sharded softmax contraction). b_out rides in core 0's waug row scaled by
the denominator, so the host division leaves it intact.

Engine plan (all three softmax-exp paths produce expT = exp(s+b) tiles):
  - scores are computed pre-scaled by A = 128*log2(e) (folded into w3's q
    columns), so PSUM holds A*s.
  - "sch"   : DVE adds an int16 C-table (A*b + 127*128 - c0) to A*s and emits
              int16 bits that ARE bf16 exp(s+b) (Schraudolph bit-trick).
  - "peadd" : PE accumulates an f32r C-table onto the score PSUM via an
              identity matmul; ACT computes exp(scale*x + bias) exactly.
  - "act"   : ACT computes exp(s); DVE or GPSIMD multiplies by exp(b) slabs.
  The three paths split the 64 exp-units across DVE/PE+ACT/ACT+GPSIMD so no
  single engine owns the softmax.

PV is computed transposed (lhsT = expT[j, i], rhs = v[j, 33]) so each matmul
streams only 33 columns; the per-i-tile [i, dh] accumulators are transposed
back to [dh, i] with PE-transpose and projected as out[c, i] = waug.T @ outT.

The emission is software-pipelined with a configurable PV lag and a 5-stage
chunk epilogue so the in-order PE stream never waits on a fresh cross-engine
result. PSUM (8 banks): 3x score [128,1024] (6) + chunk-parity accumulator
pair [128,264] (1) + a shared transpose/projection bank (1); the accumulator
uses DVE memset + start=False accumulation (per-element has_written) because
any start=True matmul clears the other slices sharing its bank.

Bias trick: with n ordered z-major, the [2048, 2048] per-head bias matrix is
block-Toeplitz over z with 256x256 blocks indexed by dz = zi - zj; only 7
slabs (k = ic - t + 3) are needed per (i-chunk, j-block) pair.
"""

import os
import sys

import numpy as np

for _p in ("/opt/trn_rl_repo", "/root/.axon_site/_ro/trn_rl_repo"):
    if os.path.isdir(_p) and _p not in sys.path:
        sys.path.append(_p)

import ml_dtypes  # noqa: E402
import concourse.bass as bass  # noqa: E402
import concourse.tile as tile  # noqa: E402
from concourse import bacc, mybir  # noqa: E402
from concourse.bass_utils import run_bass_kernel_spmd  # noqa: E402

BF16 = mybir.dt.bfloat16
F32 = mybir.dt.float32
F32R = mybir.dt.float32r
I16 = mybir.dt.int16
NPBF16 = ml_dtypes.bfloat16

B = 2            # batches
N = 2048         # tokens per batch (= 8*16*16, z-major)
C = 128          # channels
HEADS = 8
DH = 32          # head dim
NCORES = 8

A_SCALE = 128.0 * np.log2(np.e)        # Schraudolph prescale (bf16 bit trick)
BC_CONST = 127.0 * 128.0 - 7.0         # Schraudolph offset, c0 = 7 calibrated

# PV-T trails scores by LAG steps. NOTE: LAG=4 produces a deterministic
# wrong result (untraced emission coincidence); 2/3/5/6 are verified correct.
LAG = int(os.environ.get("KLAG", "7"))

# ---------------------------------------------------------------------------
# static unit-path assignment: (b, ic, t, hh) -> exp path
# k = ic - t + 3 selects the bias slab; counts per k: [4,8,12,16,12,8,4]
# ---------------------------------------------------------------------------


# slot map (b, t, hh) -> path; temporally de-clustered so consecutive steps
# alternate DVE-sch (hh=0) with ACT-exp (hh=1) and no engine sees a burst.
# counts: sch 28 (DVE), act 28 (ACT + GP/DVE mult), peadd 8 (PE+ACT).
_SLOT_PATH = {}
for _b in range(B):
    for _t in range(4):
        _SLOT_PATH[(_b, _t, 0)] = "sch"
        _SLOT_PATH[(_b, _t, 1)] = "act"
_SLOT_PATH[(1, 0, 0)] = "act"      # give DVE a breather slot -> ACT
_SLOT_PATH[(0, 3, 1)] = "peadd"    # cf32 k0..3 staggered over chunks
_SLOT_PATH[(1, 3, 1)] = "peadd"
_SLOT_PATH[(0, 2, 1)] = "peadd"    # adds cf32 k4


def unit_path(b, ic, t, hh):
    return _SLOT_PATH[(b, t, hh)]


GP_MULT_FRAC = (3, 4)       # 3 of every 4 "act" multiplies go to GPSIMD

STEPS = [(b, ic, t, hh)
         for b in range(B) for ic in range(4)
         for t in range(4) for hh in range(2)]

# ---------------------------------------------------------------------------
# host-side static index table for the bias slabs
# bias7[p, k*2048 + g*512 + ih] pairs with the scores^T tile for the step
# with chunk/group offset k = ic - t + 3:
#   scores^T[j, i] tile with j = (4t+g)*128 + p, i = ic*512 + ih.
# biasT[j, i] = T[(zi-zj+7)*961 + (dy+15)*31 + (dx+15)]
# ---------------------------------------------------------------------------


def _bias7_index() -> np.ndarray:
    kk = np.arange(7)[:, None, None, None]
    gg = np.arange(4)[None, :, None, None]
    pp = np.arange(128)[None, None, :, None]
    ii = np.arange(512)[None, None, None, :]
    a = 2 * kk + 1 + ii // 256 - gg // 2          # zi - zj + 7
    pj = (gg % 2) * 128 + pp
    pi = ii % 256
    dy = pi // 16 - pj // 16 + 15
    dx = pi % 16 - pj % 16 + 15
    return (a * 961 + dy * 31 + dx).astype(np.int32)  # [7, 4, 128, 512]


_IDX7 = _bias7_index()

# ---------------------------------------------------------------------------
# device graph
# ---------------------------------------------------------------------------


def _build():
    nc = bacc.Bacc(None, target_bir_lowering=False, debug=False)

    xt_e = nc.declare_dram_parameter("xt", [C, B * N], BF16, isOutput=False)
    w3_e = nc.declare_dram_parameter("w3", [C, 96], BF16, isOutput=False)
    waug_e = nc.declare_dram_parameter("waug", [DH + 1, C], BF16, isOutput=False)
    idb_e = nc.declare_dram_parameter("idb", [128, 128], BF16, isOutput=False)
    idf_e = nc.declare_dram_parameter("idf", [128, 128], F32R, isOutput=False)
    expb_e = nc.declare_dram_parameter(
        "expb", [128, 7 * 2048], BF16, isOutput=False)
    cint_e = nc.declare_dram_parameter(
        "cint", [128, 7 * 1024], I16, isOutput=False)
    cf32_e = nc.declare_dram_parameter(
        "cf32", [128, 5 * 1024], F32R, isOutput=False)
    out_e = nc.declare_dram_parameter("out", [C, B * N], BF16, isOutput=True)
    den_e = nc.declare_dram_parameter("den", [8, 512], BF16, isOutput=True)


    with tile.TileContext(nc) as tc:
        with tc.tile_pool(name="persist", bufs=1) as persist:
            # phase-1-critical loads first
            w3 = persist.tile([C, 96], BF16)
            nc.sync.dma_start(w3[:], w3_e[:])
            xt = persist.tile([C, B * N], BF16)
            xt_dma = [nc.sync.dma_start(xt[:, b * N:(b + 1) * N],
                                        xt_e[:, b * N:(b + 1) * N])
                      for b in range(B)]
            expb = persist.tile([128, 7 * 2048], BF16)
            cint = persist.tile([128, 7 * 1024], I16)
            cf32 = persist.tile([128, 5 * 1024], F32R)
            waug = persist.tile([DH + 1, C], BF16)
            idb = persist.tile([128, 128], BF16)
            idf = persist.tile([128, 128], F32R)

            # background loads in first-use order: (table, k, hh) half-slabs
            # plus idf/idb/waug keyed by their first-use step. No chaining:
            # pure loads have no deps, so SP issues back-to-back and the DMA
            # engines drain them in this order.
            first_use = {}
            for s, (b, ic, t, hh) in enumerate(STEPS):
                k = ic - t + 3
                p = unit_path(b, ic, t, hh)
                if p == "sch":
                    key, item = ("ci", k), (s, cint, cint_e, k * 1024, 1024)
                elif p == "peadd":
                    key, item = ("cf", k), (s, cf32, cf32_e, k * 1024, 1024)
                    if ("idf",) not in first_use:
                        first_use[("idf",)] = (s - 2, idf, idf_e, 0, 128)
                else:
                    key = ("eb", k, hh)
                    item = (s, expb, expb_e, k * 2048 + hh * 1024, 1024)
                if key not in first_use:
                    first_use[key] = item
            first_use[("idb",)] = (7 + LAG + 2, idb, idb_e, 0, 128)
            first_use[("waug",)] = (7 + LAG + 4, waug, waug_e, 0, C)
            loads = sorted(first_use.values(), key=lambda v: v[0])
            # the first few slabs must beat the first steps; the rest are
            # emitted after phase 1 so the k-shift DMAs (which gate every
            # score matmul) get the DMA engines first.
            for _, tb, e_, c0, w_ in loads[:4]:
                nc.sync.dma_start(tb[:, c0:c0 + w_], e_[:, c0:c0 + w_])

            nbias = persist.tile([128, 1], F32)
            nc.vector.memset(nbias[:], -BC_CONST / A_SCALE)
            scratch = persist.tile([128, 1], F32)
            nc.vector.memset(scratch[:], 0.0)

            # persistent phase-1 outputs
            qk_sb = persist.tile([64, B * N], BF16, name="qk_sb")  # q rows 0:32
            k_sb = persist.tile([32, B * N], BF16, name="k_sb")
            vaug = [persist.tile([128, 16 * 33], BF16, tag=f"vaug{b}",
                                 name=f"vaug{b}") for b in range(B)]

            # ---- phase 1: qkv projections -------------------------------
            # batch-0 qk evicts ride ACT, batch-1 DVE, so the k-shift DMA for
            # b0 (which gates the whole main loop) is ready ASAP and ACT is
            # free for the Exp table warmup before the first real exp.
            with tc.tile_pool(name="ph1", bufs=2, space="PSUM") as ph1:
                for b in range(B):
                    nc.gpsimd.memset(vaug[b][:], 1.0)
                    for ch in range(2):
                        qk_ps = ph1.tile([64, 1024], F32, tag="qk_ps")
                        for u in range(2):
                            nc.tensor.matmul(
                                qk_ps[:, u * 512:(u + 1) * 512],
                                lhsT=w3[:, 0:64],
                                rhs=xt[:, b * N + ch * 1024 + u * 512:
                                       b * N + ch * 1024 + (u + 1) * 512],
                                start=True, stop=True)
                        dst = qk_sb[:, b * N + ch * 1024:b * N + (ch + 1) * 1024]
                        if b == 0:
                            nc.scalar.copy(dst, qk_ps[:])
                        else:
                            nc.vector.tensor_copy(dst, qk_ps[:])
                        # k rows 32:64 -> partitions 0:32 (SBUF->SBUF DMA)
                        nc.sync.dma_start(
                            k_sb[:, b * N + ch * 1024:b * N + (ch + 1) * 1024],
                            qk_sb[32:64, b * N + ch * 1024:b * N + (ch + 1) * 1024])
                    if b == 0:
                        # warm the Exp table while b1 prep runs
                        nc.scalar.activation(scratch[:], scratch[:],
                                             mybir.ActivationFunctionType.Exp)
                    for tt in range(4):
                        v_ps = ph1.tile([128, 128], F32, tag="v_ps")
                        for u in range(4):
                            nt = tt * 4 + u
                            nc.tensor.matmul(v_ps[:, u * 32:(u + 1) * 32],
                                             lhsT=xt[:, b * N + nt * 128:
                                                     b * N + (nt + 1) * 128],
                                             rhs=w3[:, 64:96],
                                             start=True, stop=True)
                        dst = vaug[b][:, tt * 132:(tt + 1) * 132]
                        dst = dst.rearrange("p (f c) -> p f c", f=4)[:, :, 0:DH]
                        src = v_ps[:].rearrange("p (f c) -> p f c", f=4)
                        nc.vector.tensor_copy(dst, src)

            for _, tb, e_, c0, w_ in loads[4:]:
                nc.sync.dma_start(tb[:, c0:c0 + w_], e_[:, c0:c0 + w_])

            # ---- phase 2: attention ------------------------------------
            KSEP = int(os.environ.get("KSEP", "0"))
            SCORE_BUFS = int(os.environ.get("KSB", "3"))
            with (
                tc.tile_pool(name="score", bufs=SCORE_BUFS,
                             space="PSUM") as score_pool,
                tc.tile_pool(name="eps", bufs=1, space="PSUM") as eps,
                tc.tile_pool(name="sbS", bufs=3) as sbS,
                tc.tile_pool(name="sbT", bufs=int(os.environ.get("KTB", str(LAG + 3)))) as sbT,
                tc.tile_pool(name="sbE", bufs=2) as sbE,
            ):
                # chunk-parity accumulator pair (1 bank) + shared
                # transpose/projection bank: trn lives in the upper half of
                # the prj bank (time-disjoint; region deps order the uses).
                accp = eps.tile([128, 264], F32, name="accp", tag="accp")
                epi = eps.tile([C, 512], F32, name="epi", tag="epi")
                if KSEP:
                    trn_sep = eps.tile([DH + 1, 512], BF16, name="trnsep",
                                       tag="trnsep")
                    trn_view = trn_sep[:]
                else:
                    trn_view = epi[0:DH + 1, 256:512].bitcast(BF16)  # [33, 512]

                def emit_scores(s):
                    b, ic, t, hh = STEPS[s]
                    path = unit_path(b, ic, t, hh)
                    k7 = ic - t + 3
                    score_ps = score_pool.tile([128, 1024], F32,
                                               name="score_ps", tag="score_ps")
                    for g in range(2):
                        jt = 4 * t + 2 * hh + g
                        nc.tensor.matmul(
                            score_ps[:, g * 512:(g + 1) * 512],
                            lhsT=k_sb[:, b * N + jt * 128:b * N + (jt + 1) * 128],
                            rhs=qk_sb[0:32, b * N + ic * 512:b * N + (ic + 1) * 512],
                            start=True, stop=(path != "peadd"),
                            skip_group_check=True)
                    if path == "peadd":
                        cbase = k7 * 1024
                        for g in range(2):
                            nc.tensor.matmul(
                                score_ps[:, g * 512:(g + 1) * 512],
                                lhsT=idf[:],
                                rhs=cf32[:, cbase + g * 512:cbase + (g + 1) * 512],
                                start=False, stop=True,
                                skip_group_check=True)
                    return score_ps

                n_actmult = 0

                def emit_exp(s, score_ps):
                    nonlocal n_actmult
                    b, ic, t, hh = STEPS[s]
                    path = unit_path(b, ic, t, hh)
                    k7 = ic - t + 3
                    if path == "sch":
                        expT = sbT.tile([128, 1024], I16, tag="expTi",
                                        name="expTi")
                        with tc.high_priority(offset=PRI_EXP), \
                             nc.allow_low_precision(reason="schraudolph"):
                            nc.vector.tensor_add(
                                expT[:], score_ps[:],
                                cint[:, k7 * 1024:(k7 + 1) * 1024])
                        return expT[:].bitcast(BF16)
                    if path == "peadd":
                        expT = sbT.tile([128, 1024], BF16, tag="expTb",
                                        name="expTb")
                        with tc.high_priority(offset=PRI_EXP):
                            nc.scalar.activation(
                                expT[:], score_ps[:],
                                mybir.ActivationFunctionType.Exp,
                                bias=nbias[:], scale=1.0 / A_SCALE)
                        return expT[:]
                    expS = sbS.tile([128, 1024], BF16, tag="expS", name="expS")
                    with tc.high_priority(offset=PRI_EXP):
                        nc.scalar.activation(
                            expS[:], score_ps[:],
                            mybir.ActivationFunctionType.Exp,
                            scale=1.0 / A_SCALE)
                    expT = sbT.tile([128, 1024], BF16, tag="expTb",
                                    name="expTb")
                    eng = (nc.gpsimd
                           if n_actmult % GP_MULT_FRAC[1] < GP_MULT_FRAC[0]
                           else nc.vector)
                    n_actmult += 1
                    eng.tensor_mul(
                        expT[:], expS[:],
                        expb[:, k7 * 2048 + hh * 1024:
                             k7 * 2048 + (hh + 1) * 1024])
                    return expT[:]

                def emit_pvt(s, expT_ap):
                    b, ic, t, hh = STEPS[s]
                    half = ((b * 4 + ic) % 2) * 132
                    if (t, hh) == (0, 0):
                        nc.vector.memset(accp[:, half:half + 132], 0.0)
                    for g in range(2):
                        jt = 4 * t + 2 * hh + g
                        for it in range(4):
                            nc.tensor.matmul(
                                accp[:, half + it * 33:half + (it + 1) * 33],
                                lhsT=expT_ap[:, g * 512 + it * 128:
                                             g * 512 + (it + 1) * 128],
                                rhs=vaug[b][:, jt * 33:(jt + 1) * 33],
                                start=False,
                                stop=(t == 3 and hh == 1 and g == 1),
                                skip_group_check=True)

                PRI_EXP = int(os.environ.get("KPE", "30"))
                PRI_EPI = int(os.environ.get("KPP", "-40"))

                # 5-stage chunk epilogue, one stage per step
                def epi_stage_inner(chunk, stage):
                    b, ic = divmod(chunk, 4)
                    half = (chunk % 2) * 132
                    if stage == 1:
                        acc_sb = sbE.tile([128, 132], BF16, tag="acc_sb",
                                          name="acc_sb")
                        nc.vector.tensor_copy(acc_sb[:],
                                              accp[:, half:half + 132])
                        self_state[chunk] = acc_sb
                    elif stage == 2:
                        acc_sb = self_state[chunk]
                        for it in range(4):
                            nc.tensor.transpose(
                                trn_view[:, it * 128:(it + 1) * 128],
                                acc_sb[:, it * 33:(it + 1) * 33], idb[:])
                    elif stage == 3:
                        trn_sb = sbE.tile([DH + 1, 512], BF16, tag="trn_sb",
                                          name="trn_sb")
                        nc.vector.tensor_copy(trn_sb[:], trn_view[:])
                        nc.sync.dma_start(den_e[chunk:chunk + 1, :],
                                          trn_sb[DH:DH + 1, :])
                        self_state[chunk] = trn_sb
                    elif stage == 4:
                        trn_sb = self_state[chunk]
                        nc.tensor.matmul(epi[:], lhsT=waug[:], rhs=trn_sb[:],
                                         start=True, stop=True)
                    elif stage == 5:
                        prj_sb = sbE.tile([C, 512], BF16, tag="prj_sb",
                                          name="prj_sb")
                        nc.scalar.copy(prj_sb[:], epi[:])
                        nc.sync.dma_start(
                            out_e[:, b * N + ic * 512:b * N + (ic + 1) * 512],
                            prj_sb[:])

                def epi_stage(chunk, stage):
                    with tc.high_priority(offset=PRI_EPI):
                        epi_stage_inner(chunk, stage)

                self_state = {}
                score_of = {}
                expT_of = {}
                n_steps = len(STEPS)
                for s in range(n_steps + LAG + 14):
                    # PV-T and epilogue first: their deps are LAG steps old,
                    # so PE drains them while the fresh score matmuls wait on
                    # score-tile recycling.
                    for chunk in range(8):
                        st = s - (8 * chunk + 7 + LAG)
                        if st in (1, 3, 5):
                            epi_stage(chunk, st)
                    if 0 <= s - LAG < n_steps:
                        emit_pvt(s - LAG, expT_of.pop(s - LAG))
                    for chunk in range(8):
                        st = s - (8 * chunk + 7 + LAG)
                        if st in (2, 4):
                            epi_stage(chunk, st)
                    if s < n_steps:
                        score_of[s] = emit_scores(s)
                    if 0 <= s - 1 < n_steps:
                        expT_of[s - 1] = emit_exp(s - 1, score_of.pop(s - 1))

    nc.compile()
    return nc


_NC = None


def _get_nc():
    global _NC
    if _NC is None:
        _NC = _build()
    return _NC


# ---------------------------------------------------------------------------
# host side
# ---------------------------------------------------------------------------


def _prep_in_maps(x, w_qkv, rel_table, w_out, b_out):
    x = np.asarray(x, np.float32)
    w_qkv = np.asarray(w_qkv, np.float32)
    rel_table = np.asarray(rel_table, np.float32)
    w_out = np.asarray(w_out, np.float32)
    b_out = np.asarray(b_out, np.float32)

    scale = DH ** -0.5
    xt = np.ascontiguousarray(x.transpose(2, 0, 1).reshape(C, B * N)).astype(NPBF16)
    idb = np.eye(128, dtype=NPBF16)
    idf = np.eye(128, dtype=np.float32)

    in_maps = []
    for hc in range(NCORES):
        w3 = np.concatenate([
            w_qkv[:, hc * DH:(hc + 1) * DH] * (scale * A_SCALE),
            w_qkv[:, 256 + hc * DH: 256 + (hc + 1) * DH],
            w_qkv[:, 512 + hc * DH: 512 + (hc + 1) * DH],
        ], axis=1).astype(NPBF16)
        waug = np.zeros((DH + 1, C), np.float32)
        waug[0:DH, :] = w_out[hc * DH:(hc + 1) * DH, :]
        if hc == 0:
            waug[DH, :] = b_out
        bias7 = rel_table[:, hc][_IDX7]                    # [7, 4, 128, 512]

        def slab(k):
            return np.ascontiguousarray(
                bias7[k].transpose(1, 0, 2).reshape(128, 2048))

        expb = np.concatenate([np.exp(slab(k)) for k in range(7)],
                              axis=1).astype(NPBF16)
        cint = np.concatenate(
            [np.round(A_SCALE * slab(k)[:, 0:1024] + BC_CONST)
             for k in range(7)], axis=1).astype(np.int16)
        cf32 = np.concatenate(
            [A_SCALE * slab(k)[:, 1024:2048] + BC_CONST for k in range(5)],
            axis=1).astype(np.float32)
        in_maps.append({
            "xt": xt,
            "w3": np.ascontiguousarray(w3),
            "waug": waug.astype(NPBF16),
            "idb": idb,
            "idf": idf,
            "expb": expb,
            "cint": cint,
            "cf32": cf32,
        })
    return in_maps


def _run(in_maps, **kwargs):
    nc = _get_nc()
    return run_bass_kernel_spmd(nc, in_maps, core_ids=list(range(NCORES)), **kwargs)


def _combine(res):
    acc = np.zeros((C, B * N), np.float64)
    for i in range(NCORES):
        out = res.results[i]["out"].astype(np.float64)      # [C, B*N]
        den = res.results[i]["den"].astype(np.float64).reshape(B * N)
        acc += out / den[None, :]
    return acc.reshape(C, B, N).transpose(1, 2, 0).astype(np.float32)


def kernel(x, w_qkv, rel_table, w_out, b_out, d=None, h=None, w=None):
    in_maps = _prep_in_maps(x, w_qkv, rel_table, w_out, b_out)
    res = _run(in_maps)
    return _combine(res)


# revision 23
# speedup vs baseline: 1.0141x; 1.0141x over previous
"""Distributed Trainium2 Bass kernel for nn_AttentionD_12412455485977.

3D-windowed multi-head attention with relative-position bias:
  qkv = x @ w_qkv ; per-head attention with bias gathered from rel_table
  via the static relative-position index; out = attn_out @ w_out + b_out.

Sharding: head-parallel. Core c computes head c for both batches; each core
returns an UNNORMALIZED projected output [C, B*N] plus the per-(b, i) softmax
denominator; the host divides and sums across cores (the natural unshard of a
head-sharded softmax contraction). b_out rides in core 0's waug row scaled by
the denominator, so the host division leaves it intact.

Engine plan (all three softmax-exp paths produce expT = exp(s+b) tiles):
  - scores are computed pre-scaled by A = 128*log2(e) (folded into w3's q
    columns), so PSUM holds A*s.
  - "sch"   : DVE adds an int16 C-table (A*b + 127*128 - c0) to A*s and emits
              int16 bits that ARE bf16 exp(s+b) (Schraudolph bit-trick).
  - "peadd" : PE accumulates an f32r C-table onto the score PSUM via an
              identity matmul; ACT computes exp(scale*x + bias) exactly.
  - "act"   : ACT computes exp(s); DVE or GPSIMD multiplies by exp(b) slabs.
  The three paths split the 64 exp-units across DVE/PE+ACT/ACT+GPSIMD so no
  single engine owns the softmax.

PV is computed transposed (lhsT = expT[j, i], rhs = v[j, 33]) so each matmul
streams only 33 columns; the per-i-tile [i, dh] accumulators are transposed
back to [dh, i] with PE-transpose and projected as out[c, i] = waug.T @ outT.

The emission is software-pipelined with a configurable PV lag and a 5-stage
chunk epilogue so the in-order PE stream never waits on a fresh cross-engine
result. PSUM (8 banks): 3x score [128,1024] (6) + chunk-parity accumulator
pair [128,264] (1) + a shared transpose/projection bank (1); the accumulator
uses DVE memset + start=False accumulation (per-element has_written) because
any start=True matmul clears the other slices sharing its bank.

Bias trick: with n ordered z-major, the [2048, 2048] per-head bias matrix is
block-Toeplitz over z with 256x256 blocks indexed by dz = zi - zj; only 7
slabs (k = ic - t + 3) are needed per (i-chunk, j-block) pair.
"""

import os
import sys

import numpy as np

for _p in ("/opt/trn_rl_repo", "/root/.axon_site/_ro/trn_rl_repo"):
    if os.path.isdir(_p) and _p not in sys.path:
        sys.path.append(_p)

import ml_dtypes  # noqa: E402
import concourse.bass as bass  # noqa: E402
import concourse.tile as tile  # noqa: E402
from concourse import bacc, mybir  # noqa: E402
from concourse.bass_utils import run_bass_kernel_spmd  # noqa: E402

BF16 = mybir.dt.bfloat16
F32 = mybir.dt.float32
F32R = mybir.dt.float32r
I16 = mybir.dt.int16
NPBF16 = ml_dtypes.bfloat16

B = 2            # batches
N = 2048         # tokens per batch (= 8*16*16, z-major)
C = 128          # channels
HEADS = 8
DH = 32          # head dim
NCORES = 8

A_SCALE = 128.0 * np.log2(np.e)        # Schraudolph prescale (bf16 bit trick)
BC_CONST = 127.0 * 128.0 - 7.0         # Schraudolph offset, c0 = 7 calibrated

# PV-T trails scores by LAG steps. NOTE: LAG=4 produces a deterministic
# wrong result (untraced emission coincidence); 2/3/5/6 are verified correct.
LAG = int(os.environ.get("KLAG", "7"))

# ---------------------------------------------------------------------------
# static unit-path assignment: (b, ic, t, hh) -> exp path
# k = ic - t + 3 selects the bias slab; counts per k: [4,8,12,16,12,8,4]
# ---------------------------------------------------------------------------


# slot map (b, t, hh) -> path; temporally de-clustered so consecutive steps
# alternate DVE-sch (hh=0) with ACT-exp (hh=1) and no engine sees a burst.
# counts: sch 28 (DVE), act 28 (ACT + GP/DVE mult), peadd 8 (PE+ACT).
_SLOT_PATH = {}
for _b in range(B):
    for _t in range(4):
        _SLOT_PATH[(_b, _t, 0)] = "sch"
        _SLOT_PATH[(_b, _t, 1)] = "act"
_SLOT_PATH[(1, 0, 0)] = "act"      # give DVE a breather slot -> ACT
_SLOT_PATH[(0, 3, 1)] = "peadd"    # cf32 k0..3 staggered over chunks
_SLOT_PATH[(1, 3, 1)] = "peadd"
_SLOT_PATH[(0, 2, 1)] = "peadd"    # adds cf32 k4


def unit_path(b, ic, t, hh):
    return _SLOT_PATH[(b, t, hh)]


GP_MULT_FRAC = (3, 4)       # 3 of every 4 "act" multiplies go to GPSIMD

STEPS = [(b, ic, t, hh)
         for b in range(B) for ic in range(4)
         for t in range(4) for hh in range(2)]

# ---------------------------------------------------------------------------
# host-side static index table for the bias slabs
# bias7[p, k*2048 + g*512 + ih] pairs with the scores^T tile for the step
# with chunk/group offset k = ic - t + 3:
#   scores^T[j, i] tile with j = (4t+g)*128 + p, i = ic*512 + ih.
# biasT[j, i] = T[(zi-zj+7)*961 + (dy+15)*31 + (dx+15)]
# ---------------------------------------------------------------------------


def _bias7_index() -> np.ndarray:
    kk = np.arange(7)[:, None, None, None]
    gg = np.arange(4)[None, :, None, None]
    pp = np.arange(128)[None, None, :, None]
    ii = np.arange(512)[None, None, None, :]
    a = 2 * kk + 1 + ii // 256 - gg // 2          # zi - zj + 7
    pj = (gg % 2) * 128 + pp
    pi = ii % 256
    dy = pi // 16 - pj // 16 + 15
    dx = pi % 16 - pj % 16 + 15
    return (a * 961 + dy * 31 + dx).astype(np.int32)  # [7, 4, 128, 512]


_IDX7 = _bias7_index()

# ---------------------------------------------------------------------------
# device graph
# ---------------------------------------------------------------------------


def _build():
    nc = bacc.Bacc(None, target_bir_lowering=False, debug=False)

    xt_e = nc.declare_dram_parameter("xt", [C, B * N], BF16, isOutput=False)
    w3_e = nc.declare_dram_parameter("w3", [C, 96], BF16, isOutput=False)
    waug_e = nc.declare_dram_parameter("waug", [DH + 1, C], BF16, isOutput=False)
    idb_e = nc.declare_dram_parameter("idb", [128, 128], BF16, isOutput=False)
    idf_e = nc.declare_dram_parameter("idf", [128, 128], F32R, isOutput=False)
    expb_e = nc.declare_dram_parameter(
        "expb", [128, 7 * 2048], BF16, isOutput=False)
    cint_e = nc.declare_dram_parameter(
        "cint", [128, 7 * 1024], I16, isOutput=False)
    cf32_e = nc.declare_dram_parameter(
        "cf32", [128, 5 * 1024], F32R, isOutput=False)
    out_e = nc.declare_dram_parameter("out", [C, B * N], BF16, isOutput=True)
    den_e = nc.declare_dram_parameter("den", [8, 512], BF16, isOutput=True)


    with tile.TileContext(nc) as tc:
        with tc.tile_pool(name="persist", bufs=1) as persist:
            # phase-1-critical loads first
            w3 = persist.tile([C, 96], BF16)
            nc.sync.dma_start(w3[:], w3_e[:])
            xt = persist.tile([C, B * N], BF16)
            xt_dma = [nc.sync.dma_start(xt[:, b * N:(b + 1) * N],
                                        xt_e[:, b * N:(b + 1) * N])
                      for b in range(B)]
            expb = persist.tile([128, 7 * 2048], BF16)
            cint = persist.tile([128, 7 * 1024], I16)
            cf32 = persist.tile([128, 5 * 1024], F32R)
            waug = persist.tile([DH + 1, C], BF16)
            idb = persist.tile([128, 128], BF16)
            idf = persist.tile([128, 128], F32R)

            # background loads in first-use order: (table, k, hh) half-slabs
            # plus idf/idb/waug keyed by their first-use step. No chaining:
            # pure loads have no deps, so SP issues back-to-back and the DMA
            # engines drain them in this order.
            first_use = {}
            for s, (b, ic, t, hh) in enumerate(STEPS):
                k = ic - t + 3
                p = unit_path(b, ic, t, hh)
                if p == "sch":
                    key, item = ("ci", k), (s, cint, cint_e, k * 1024, 1024)
                elif p == "peadd":
                    key, item = ("cf", k), (s, cf32, cf32_e, k * 1024, 1024)
                    if ("idf",) not in first_use:
                        first_use[("idf",)] = (s - 2, idf, idf_e, 0, 128)
                else:
                    key = ("eb", k, hh)
                    item = (s, expb, expb_e, k * 2048 + hh * 1024, 1024)
                if key not in first_use:
                    first_use[key] = item
            first_use[("idb",)] = (7 + LAG + 2, idb, idb_e, 0, 128)
            first_use[("waug",)] = (7 + LAG + 4, waug, waug_e, 0, C)
            loads = sorted(first_use.values(), key=lambda v: v[0])
            # the first few slabs must beat the first steps; the rest are
            # emitted after phase 1 so the k-shift DMAs (which gate every
            # score matmul) get the DMA engines first.
            for _, tb, e_, c0, w_ in loads[:4]:
                nc.sync.dma_start(tb[:, c0:c0 + w_], e_[:, c0:c0 + w_])

            nbias = persist.tile([128, 1], F32)
            nc.vector.memset(nbias[:], -BC_CONST / A_SCALE)
            scratch = persist.tile([128, 1], F32)
            nc.vector.memset(scratch[:], 0.0)

            # persistent phase-1 outputs
            qk_sb = persist.tile([64, B * N], BF16, name="qk_sb")  # q rows 0:32
            k_sb = persist.tile([32, B * N], BF16, name="k_sb")
            vaug = [persist.tile([128, 16 * 33], BF16, tag=f"vaug{b}",
                                 name=f"vaug{b}") for b in range(B)]

            # ---- phase 1: qkv projections -------------------------------
            # batch-0 qk evicts ride ACT, batch-1 DVE, so the k-shift DMA for
            # b0 (which gates the whole main loop) is ready ASAP and ACT is
            # free for the Exp table warmup before the first real exp.
            with tc.tile_pool(name="ph1", bufs=2, space="PSUM") as ph1:
                for b in range(B):
                    nc.gpsimd.memset(vaug[b][:], 1.0)
                    for ch in range(2):
                        qk_ps = ph1.tile([64, 1024], F32, tag="qk_ps")
                        for u in range(2):
                            nc.tensor.matmul(
                                qk_ps[:, u * 512:(u + 1) * 512],
                                lhsT=w3[:, 0:64],
                                rhs=xt[:, b * N + ch * 1024 + u * 512:
                                       b * N + ch * 1024 + (u + 1) * 512],
                                start=True, stop=True)
                        dst = qk_sb[:, b * N + ch * 1024:b * N + (ch + 1) * 1024]
                        if b == 0:
                            nc.scalar.copy(dst, qk_ps[:])
                        else:
                            nc.vector.tensor_copy(dst, qk_ps[:])
                        # k rows 32:64 -> partitions 0:32 (SBUF->SBUF DMA)
                        nc.sync.dma_start(
                            k_sb[:, b * N + ch * 1024:b * N + (ch + 1) * 1024],
                            qk_sb[32:64, b * N + ch * 1024:b * N + (ch + 1) * 1024])
                    if b == 0:
                        # warm the Exp table while b1 prep runs
                        nc.scalar.activation(scratch[:], scratch[:],
                                             mybir.ActivationFunctionType.Exp)
                    for tt in range(4):
                        v_ps = ph1.tile([128, 128], F32, tag="v_ps")
                        for u in range(4):
                            nt = tt * 4 + u
                            nc.tensor.matmul(v_ps[:, u * 32:(u + 1) * 32],
                                             lhsT=xt[:, b * N + nt * 128:
                                                     b * N + (nt + 1) * 128],
                                             rhs=w3[:, 64:96],
                                             start=True, stop=True)
                        dst = vaug[b][:, tt * 132:(tt + 1) * 132]
                        dst = dst.rearrange("p (f c) -> p f c", f=4)[:, :, 0:DH]
                        src = v_ps[:].rearrange("p (f c) -> p f c", f=4)
                        nc.vector.tensor_copy(dst, src)

            for _, tb, e_, c0, w_ in loads[4:]:
                nc.sync.dma_start(tb[:, c0:c0 + w_], e_[:, c0:c0 + w_])

            # ---- phase 2: attention ------------------------------------
            KSEP = int(os.environ.get("KSEP", "0"))
            SCORE_BUFS = int(os.environ.get("KSB", "3"))
            with (
                tc.tile_pool(name="score", bufs=SCORE_BUFS,
                             space="PSUM") as score_pool,
                tc.tile_pool(name="eps", bufs=1, space="PSUM") as eps,
                tc.tile_pool(name="sbS", bufs=3) as sbS,
                tc.tile_pool(name="sbT", bufs=int(os.environ.get("KTB", str(LAG + 3)))) as sbT,
                tc.tile_pool(name="sbE", bufs=int(os.environ.get("KEB", "2"))) as sbE,
            ):
                # chunk-parity accumulator pair (1 bank) + shared
                # transpose/projection bank: trn lives in the upper half of
                # the prj bank (time-disjoint; region deps order the uses).
                accp = eps.tile([128, 264], F32, name="accp", tag="accp")
                epi = eps.tile([C, 512], F32, name="epi", tag="epi")
                if KSEP:
                    trn_sep = eps.tile([DH + 1, 512], BF16, name="trnsep",
                                       tag="trnsep")
                    trn_view = trn_sep[:]
                else:
                    trn_view = epi[0:DH + 1, 256:512].bitcast(BF16)  # [33, 512]

                def emit_scores(s):
                    b, ic, t, hh = STEPS[s]
                    path = unit_path(b, ic, t, hh)
                    k7 = ic - t + 3
                    score_ps = score_pool.tile([128, 1024], F32,
                                               name="score_ps", tag="score_ps")
                    for g in range(2):
                        jt = 4 * t + 2 * hh + g
                        nc.tensor.matmul(
                            score_ps[:, g * 512:(g + 1) * 512],
                            lhsT=k_sb[:, b * N + jt * 128:b * N + (jt + 1) * 128],
                            rhs=qk_sb[0:32, b * N + ic * 512:b * N + (ic + 1) * 512],
                            start=True, stop=(path != "peadd"),
                            skip_group_check=True)
                    if path == "peadd":
                        cbase = k7 * 1024
                        for g in range(2):
                            nc.tensor.matmul(
                                score_ps[:, g * 512:(g + 1) * 512],
                                lhsT=idf[:],
                                rhs=cf32[:, cbase + g * 512:cbase + (g + 1) * 512],
                                start=False, stop=True,
                                skip_group_check=True)
                    return score_ps

                n_actmult = 0

                def emit_exp(s, score_ps):
                    nonlocal n_actmult
                    b, ic, t, hh = STEPS[s]
                    path = unit_path(b, ic, t, hh)
                    k7 = ic - t + 3
                    if path == "sch":
                        expT = sbT.tile([128, 1024], I16, tag="expTi",
                                        name="expTi")
                        with tc.high_priority(offset=PRI_EXP), \
                             nc.allow_low_precision(reason="schraudolph"):
                            nc.vector.tensor_add(
                                expT[:], score_ps[:],
                                cint[:, k7 * 1024:(k7 + 1) * 1024])
                        return expT[:].bitcast(BF16)
                    if path == "peadd":
                        expT = sbT.tile([128, 1024], BF16, tag="expTb",
                                        name="expTb")
                        with tc.high_priority(offset=PRI_EXP):
                            nc.scalar.activation(
                                expT[:], score_ps[:],
                                mybir.ActivationFunctionType.Exp,
                                bias=nbias[:], scale=1.0 / A_SCALE)
                        return expT[:]
                    expS = sbS.tile([128, 1024], BF16, tag="expS", name="expS")
                    with tc.high_priority(offset=PRI_EXP):
                        nc.scalar.activation(
                            expS[:], score_ps[:],
                            mybir.ActivationFunctionType.Exp,
                            scale=1.0 / A_SCALE)
                    expT = sbT.tile([128, 1024], BF16, tag="expTb",
                                    name="expTb")
                    eng = (nc.gpsimd
                           if n_actmult % GP_MULT_FRAC[1] < GP_MULT_FRAC[0]
                           else nc.vector)
                    n_actmult += 1
                    eng.tensor_mul(
                        expT[:], expS[:],
                        expb[:, k7 * 2048 + hh * 1024:
                             k7 * 2048 + (hh + 1) * 1024])
                    return expT[:]

                def emit_pvt(s, expT_ap):
                    b, ic, t, hh = STEPS[s]
                    half = ((b * 4 + ic) % 2) * 132
                    if (t, hh) == (0, 0):
                        nc.vector.memset(accp[:, half:half + 132], 0.0)
                    for g in range(2):
                        jt = 4 * t + 2 * hh + g
                        for it in range(4):
                            nc.tensor.matmul(
                                accp[:, half + it * 33:half + (it + 1) * 33],
                                lhsT=expT_ap[:, g * 512 + it * 128:
                                             g * 512 + (it + 1) * 128],
                                rhs=vaug[b][:, jt * 33:(jt + 1) * 33],
                                start=False,
                                stop=(t == 3 and hh == 1 and g == 1),
                                skip_group_check=True)

                PRI_EXP = int(os.environ.get("KPE", "30"))
                PRI_EPI = int(os.environ.get("KPP", "-40"))

                # 5-stage chunk epilogue, one stage per step
                def epi_stage_inner(chunk, stage):
                    b, ic = divmod(chunk, 4)
                    half = (chunk % 2) * 132
                    if stage == 1:
                        acc_sb = sbE.tile([128, 132], BF16, tag="acc_sb",
                                          name="acc_sb")
                        nc.vector.tensor_copy(acc_sb[:],
                                              accp[:, half:half + 132])
                        self_state[chunk] = acc_sb
                    elif stage == 2:
                        acc_sb = self_state[chunk]
                        for it in range(4):
                            nc.tensor.transpose(
                                trn_view[:, it * 128:(it + 1) * 128],
                                acc_sb[:, it * 33:(it + 1) * 33], idb[:])
                    elif stage == 3:
                        trn_sb = sbE.tile([DH + 1, 512], BF16, tag="trn_sb",
                                          name="trn_sb")
                        nc.vector.tensor_copy(trn_sb[:], trn_view[:])
                        nc.sync.dma_start(den_e[chunk:chunk + 1, :],
                                          trn_sb[DH:DH + 1, :])
                        self_state[chunk] = trn_sb
                    elif stage == 4:
                        trn_sb = self_state[chunk]
                        nc.tensor.matmul(epi[:], lhsT=waug[:], rhs=trn_sb[:],
                                         start=True, stop=True)
                    elif stage == 5:
                        prj_sb = sbE.tile([C, 512], BF16, tag="prj_sb",
                                          name="prj_sb")
                        nc.scalar.copy(prj_sb[:], epi[:])
                        nc.sync.dma_start(
                            out_e[:, b * N + ic * 512:b * N + (ic + 1) * 512],
                            prj_sb[:])

                def epi_stage(chunk, stage):
                    with tc.high_priority(offset=PRI_EPI):
                        epi_stage_inner(chunk, stage)

                self_state = {}
                score_of = {}
                expT_of = {}
                n_steps = len(STEPS)
                for s in range(n_steps + LAG + 14):
                    if s < n_steps:
                        score_of[s] = emit_scores(s)
                    if 0 <= s - 1 < n_steps:
                        expT_of[s - 1] = emit_exp(s - 1, score_of.pop(s - 1))
                    # PE epilogue stage 2/4 go after this step's PV-T; the
                    # rest can interleave anywhere on their engines.
                    for chunk in range(8):
                        st = s - (8 * chunk + 7 + LAG)
                        if st in (1, 3, 5):
                            epi_stage(chunk, st)
                    if 0 <= s - LAG < n_steps:
                        emit_pvt(s - LAG, expT_of.pop(s - LAG))
                    for chunk in range(8):
                        st = s - (8 * chunk + 7 + LAG)
                        if st in (2, 4):
                            epi_stage(chunk, st)

    nc.compile()
    return nc


_NC = None


def _get_nc():
    global _NC
    if _NC is None:
        _NC = _build()
    return _NC


# ---------------------------------------------------------------------------
# host side
# ---------------------------------------------------------------------------


def _prep_in_maps(x, w_qkv, rel_table, w_out, b_out):
    x = np.asarray(x, np.float32)
    w_qkv = np.asarray(w_qkv, np.float32)
    rel_table = np.asarray(rel_table, np.float32)
    w_out = np.asarray(w_out, np.float32)
    b_out = np.asarray(b_out, np.float32)

    scale = DH ** -0.5
    xt = np.ascontiguousarray(x.transpose(2, 0, 1).reshape(C, B * N)).astype(NPBF16)
    idb = np.eye(128, dtype=NPBF16)
    idf = np.eye(128, dtype=np.float32)

    in_maps = []
    for hc in range(NCORES):
        w3 = np.concatenate([
            w_qkv[:, hc * DH:(hc + 1) * DH] * (scale * A_SCALE),
            w_qkv[:, 256 + hc * DH: 256 + (hc + 1) * DH],
            w_qkv[:, 512 + hc * DH: 512 + (hc + 1) * DH],
        ], axis=1).astype(NPBF16)
        waug = np.zeros((DH + 1, C), np.float32)
        waug[0:DH, :] = w_out[hc * DH:(hc + 1) * DH, :]
        if hc == 0:
            waug[DH, :] = b_out
        bias7 = rel_table[:, hc][_IDX7]                    # [7, 4, 128, 512]

        def slab(k):
            return np.ascontiguousarray(
                bias7[k].transpose(1, 0, 2).reshape(128, 2048))

        expb = np.concatenate([np.exp(slab(k)) for k in range(7)],
                              axis=1).astype(NPBF16)
        cint = np.concatenate(
            [np.round(A_SCALE * slab(k)[:, 0:1024] + BC_CONST)
             for k in range(7)], axis=1).astype(np.int16)
        cf32 = np.concatenate(
            [A_SCALE * slab(k)[:, 1024:2048] + BC_CONST for k in range(5)],
            axis=1).astype(np.float32)
        in_maps.append({
            "xt": xt,
            "w3": np.ascontiguousarray(w3),
            "waug": waug.astype(NPBF16),
            "idb": idb,
            "idf": idf,
            "expb": expb,
            "cint": cint,
            "cf32": cf32,
        })
    return in_maps


def _run(in_maps, **kwargs):
    nc = _get_nc()
    return run_bass_kernel_spmd(nc, in_maps, core_ids=list(range(NCORES)), **kwargs)


def _combine(res):
    acc = np.zeros((C, B * N), np.float64)
    for i in range(NCORES):
        out = res.results[i]["out"].astype(np.float64)      # [C, B*N]
        den = res.results[i]["den"].astype(np.float64).reshape(B * N)
        acc += out / den[None, :]
    return acc.reshape(C, B, N).transpose(1, 2, 0).astype(np.float32)


def kernel(x, w_qkv, rel_table, w_out, b_out, d=None, h=None, w=None):
    in_maps = _prep_in_maps(x, w_qkv, rel_table, w_out, b_out)
    res = _run(in_maps)
    return _combine(res)
